# revision 26
# baseline (speedup 1.0000x reference)
"""GQA attention (tanh-score + static bias, no softmax) on 8 trn2 cores.

Reference shapes: x [4,32,256,512], H=8 heads, G=2 kv groups, D=64, N=256.
Strategy: data-parallel over the 128 (b,t) pairs -> 16 per core, zero
collectives.  All SBUF operands fp16 (1 cycle/row matmuls at any free size,
FWL weight loads, half the DMA of fp32), PSUM accumulation fp32.

Per (b,t):
  q^T = Wq^T x^T, k^T = Wk^T x^T   (feature-major, host-transposed x)
  v   = x @ Wv                      (token-major, lhsT = x^T chunks)
  sv^T = (sgr @ v)^T                (lhsT = v, rhs = sgr^T; both groups at once)
  scores^T[m,n] = k_g q_h^T         (K=64 contraction -> ROW-TILED: the two kv
                                     groups run concurrently in array halves;
                                     each mm streams TWO same-group heads in
                                     one N=512 rhs via a 2-block AP over the
                                     iteration's merged q tile)
  attn^T = tanh(scores^T * 0.125)   (ACT engine, scale fused, [128,1024]
                                     two-bank tiles to amortize overhead)
  out_h^T = v_g^T attn_h^T + sv^T   (COL-TILED: heads h and h+4 in col halves,
                                     zero wasted array; sv add fused into the
                                     PSUM->SBUF evacuation on DVE; quad outputs
                                     double-buffered through a PSUM pool shared
                                     with the v/sv banks)
  y = out @ Wo                      (token-major y, Wo host-permuted)

Host-side prep: x fp16 feature-major pre-tiled; Wq columns permuted to head
pairs (h, h+4) so each q tile holds one head per array half matching the k
group layout (no doubled Wk needed); Wo rows same permutation; sgr transposed.
"""

import os
import sys

import numpy as np

for _p in ("/opt/trn_rl_repo",):
    if _p not in sys.path and os.path.isdir(_p):
        sys.path.insert(0, _p)

import concourse.bass as bass
import concourse.tile as tile
from concourse import bacc, mybir
from concourse.bass_utils import run_bass_kernel_spmd

F32 = mybir.dt.float32
F16 = mybir.dt.float16

B, T, N, C = 4, 32, 256, 512
H, G, D = 8, 2, 64
NCORES = 8
BT = B * T                      # 128
PER_CORE = BT // NCORES         # 16
NPAIR = PER_CORE // 2           # 8 iterations of 2 (b,t) each
SCALE = D ** -0.5               # 0.125

_cached = {}


def _build_nc():
    """Build + lower the single-core SPMD program."""
    nc = bacc.Bacc("TRN2", target_bir_lowering=False, debug=False,
                   num_devices=NCORES)

    # DRAM I/O (per-core shard, host-side pre-arranged, fp16)
    # xT[i, p, c, 256*b + n] = x[bt=2i+b, tok=n, cin=128c+p]
    xT = nc.dram_tensor("xT", [NPAIR, 128, 4, 512], F16, kind="ExternalInput").ap()
    sgrT = nc.dram_tensor("sgrT", [N, N], F16, kind="ExternalInput").ap()
    Wqp = nc.dram_tensor("Wqp", [C, C], F16, kind="ExternalInput").ap()
    Wk = nc.dram_tensor("Wk", [C, G * D], F16, kind="ExternalInput").ap()
    Wv = nc.dram_tensor("Wv", [C, G * D], F16, kind="ExternalInput").ap()
    Wop = nc.dram_tensor("Wop", [C, C], F16, kind="ExternalInput").ap()
    y = nc.dram_tensor("y", [PER_CORE, N, C], F16, kind="ExternalOutput").ap()

    with tile.TileContext(nc) as tc:
        _body(tc, xT, sgrT, Wqp, Wk, Wv, Wop, y)

    nc.compile()
    return nc


def _body(tc, xT, sgrT, Wqp, Wk, Wv, Wop, y):
    nc = tc.nc
    mm = nc.tensor.matmul
    import contextlib
    ctx = contextlib.ExitStack()
    with ctx:
        # SBUF state pools one deeper than strictly needed (bufs=3): the next
        # iteration's projection tiles become allocatable early, so the
        # scheduler always has ready PE work to fill attention-chain stalls.
        consts = ctx.enter_context(tc.tile_pool(name="consts", bufs=1))
        xpool = ctx.enter_context(tc.tile_pool(name="xt", bufs=4))
        qpool = ctx.enter_context(tc.tile_pool(name="qs", bufs=3))
        kpool = ctx.enter_context(tc.tile_pool(name="ks", bufs=3))
        vpool = ctx.enter_context(tc.tile_pool(name="vs", bufs=3))
        svpool = ctx.enter_context(tc.tile_pool(name="svs", bufs=3))
        apool = ctx.enter_context(tc.tile_pool(name="attn", bufs=8))
        ppool = ctx.enter_context(tc.tile_pool(name="pairs", bufs=6))
        ypool = ctx.enter_context(tc.tile_pool(name="ys", bufs=4))
        # PSUM: 8 banks of [128, 512] fp32.
        #   psA 2 banks (q / k / y cycling), psS 2x two-bank tiles (scores),
        #   psVP 2 banks shared by v, sv and the four attn@v quads so the
        #   quads double-buffer instead of serializing through one bank.
        psA = ctx.enter_context(
            tc.tile_pool(name="psA", bufs=2, space=bass.MemorySpace.PSUM))
        psS = ctx.enter_context(
            tc.tile_pool(name="psS", bufs=2, space=bass.MemorySpace.PSUM))
        psVP = ctx.enter_context(
            tc.tile_pool(name="psVP", bufs=2, space=bass.MemorySpace.PSUM))

        # ---- per-iteration x prefetch (issued ahead of the consts so the
        # first q matmuls have both operands as early as possible) ----
        xts = [None] * NPAIR

        def fetch_x(it, split=False):
            t = xpool.tile([128, 4, 512], F16, tag="xt")
            if split:
                # two transfers so the first q matmul (which only needs
                # chunk c=0) can start before the whole tile lands
                nc.sync.dma_start(t[:, 0:1, :], xT[it, :, 0:1, :])
                nc.sync.dma_start(t[:, 1:4, :], xT[it, :, 1:4, :])
            else:
                nc.sync.dma_start(t[:], xT[it])
            xts[it] = t

        fetch_x(0, split=True)

        # ---- resident constants (x of iteration 1 is fetched before the Wo
        # blocks: it is needed earlier than the output weights) ----
        wq = []
        wk = []
        wv = []
        wo = []
        for c in range(4):
            t = consts.tile([128, 512], F16, tag=f"wq{c}")
            nc.sync.dma_start(t[:], Wqp[128 * c:128 * (c + 1), :])
            wq.append(t)
        for c in range(4):
            t = consts.tile([128, 128], F16, tag=f"wk{c}")
            nc.sync.dma_start(t[:], Wk[128 * c:128 * (c + 1), :])
            wk.append(t)
            t = consts.tile([128, 128], F16, tag=f"wv{c}")
            nc.sync.dma_start(t[:], Wv[128 * c:128 * (c + 1), :])
            wv.append(t)
        sgt = []
        for mc in range(2):
            t = consts.tile([128, 256], F16, tag=f"sgt{mc}")
            nc.sync.dma_start(t[:], sgrT[128 * mc:128 * (mc + 1), :])
            sgt.append(t)
        fetch_x(1)
        for c in range(4):
            t = consts.tile([128, 512], F16, tag=f"wo{c}")
            nc.sync.dma_start(t[:], Wop[128 * c:128 * (c + 1), :])
            wo.append(t)

        # ---- warm-up while the first DMAs are in flight:
        #  * a tiny tanh so the ACT table set loads before the first real
        #    activation instead of serializing the first attention chain
        #  * dense dummy matmuls so the PE clock gate reaches 8/8 (needs
        #    ~3.4us of sustained busy) before real work starts.
        dummy = consts.tile([128, 512], F16, tag="dummy")
        nc.gpsimd.memset(dummy[:], 0.0)
        dtanh = consts.tile([128, 16], F16, tag="dtanh")
        nc.scalar.activation(dtanh[:], dummy[:, 0:16],
                             mybir.ActivationFunctionType.Tanh, scale=SCALE)
        wps = psVP.tile([128, 512], F32, tag="psVP")
        for _ in range(10):
            mm(wps[:], dummy[:, 0:128], dummy[:], start=True, stop=True)

        # per-iteration state handed between pipeline stages
        state = [None] * NPAIR

        # ---------- stage A pieces (projections for iteration it) ----------
        def proj_q(it, qall, j):
            # writes the j-th 512-col chunk of the iteration's merged q tile;
            # consecutive psA groups alternate ACT/DVE evacuation so both
            # banks drain concurrently (halves the bank-recycle stall)
            xt = xts[it]
            ps = psA.tile([128, 512], F32, tag="psA")
            for c in range(4):
                mm(ps[:], wq[c][:, 128 * j:128 * (j + 1)], xt[:, c, :],
                   start=(c == 0), stop=(c == 3))
            nc.vector.tensor_copy(qall[:, j, :], ps[:])

        def proj_k(it):
            xt = xts[it]
            ps = psA.tile([128, 512], F32, tag="psA")
            for c in range(4):
                mm(ps[:], wk[c][:], xt[:, c, :],
                   start=(c == 0), stop=(c == 3))
            s = kpool.tile([128, 512], F16, tag="ks")
            nc.vector.tensor_copy(s[:], ps[:])
            return s

        def proj_v(it):
            # token-major v: block 2b+mc at cols 128*(2b+mc) holds
            # v[tok chunk mc of bt b, (g0 d | g1 d)]
            xt = xts[it]
            ps = psVP.tile([128, 512], F32, tag="psVP")
            for blk in range(4):
                b, mcc = blk // 2, blk % 2
                off = 256 * b + 128 * mcc
                for c in range(4):
                    mm(ps[:, 128 * blk:128 * (blk + 1)],
                       xt[:, c, off:off + 128], wv[c][:],
                       start=(c == 0), stop=(c == 3))
            s = vpool.tile([128, 512], F16, tag="vs")
            nc.vector.tensor_copy(s[:], ps[:])
            return s

        def proj_sv(it, v_sb):
            # sv^T[dpair, n] for both bt: (sgr@v_g)^T rows 64g:64g+64
            ps = psVP.tile([128, 512], F32, tag="psVP")
            for b in range(2):
                for mc in range(2):
                    mm(ps[:, 256 * b:256 * (b + 1)],
                       v_sb[:, 128 * (2 * b + mc):128 * (2 * b + mc + 1)],
                       sgt[mc][:], start=(mc == 0), stop=(mc == 1))
            s = svpool.tile([128, 512], F32, tag="svs")
            nc.vector.tensor_copy(s[:], ps[:])
            return s

        # ---------- stage B pieces (attention for iteration it, bt b) ------
        def scores_quad(qall, ks, b, j0):
            # one two-bank psum tile per m-chunk holding all four heads of
            # the quad: [h_j0 | h_j0+1 | h_j0+4 | h_j0+5] (one 256-col block
            # each).  Each mm streams both same-group heads in one N=512 rhs
            # (2-block AP over the merged q tile); the two group mms
            # alternate array row halves 0:64 / 64:128 and dual-issue.
            outs = []
            for mc in range(2):
                off = 256 * b + 128 * mc
                ps = psS.tile([128, 1024], F32, tag="psS")
                # the scores->tanh loop through the two psS tiles is the
                # serial heart of the kernel: prioritize it so a freed psS
                # tile is refilled ahead of any waiting projection matmuls
                with tc.high_priority(offset=40):
                    mm(ps[:, 0:512],
                       ks[0:64, off:off + 128],
                       qall[0:64, j0:j0 + 2, 256 * b:256 * (b + 1)],
                       start=True, stop=True)
                    mm(ps[:, 512:1024],
                       ks[64:128, off:off + 128],
                       qall[64:128, j0:j0 + 2, 256 * b:256 * (b + 1)],
                       start=True, stop=True)
                    a = apool.tile([128, 1024], F16, tag="attn")
                    nc.scalar.activation(a[:], ps[:],
                                         mybir.ActivationFunctionType.Tanh,
                                         scale=SCALE)
                outs.append(a)
            return outs

        def attnv_quad(v_sb, sv_sb, b, a0, a1):
            # col-tiled fat matmuls: rows 0:64 = group-0 head pair, rows
            # 64:128 = group-1 head pair, each a single 512-free matmul per
            # m-chunk covering both heads (adjacent in the a tile).  Output
            # quadrants: (0:64, 0:256)=h_even^T g0, (0:64, 256:512)=h_odd^T,
            # (64:128, *) same for group 1 -> col 256-blocks are the
            # (h, h+4) pairs that Wop expects.
            ps = psVP.tile([128, 512], F32, tag="psVP")
            for mc, a in ((0, a0), (1, a1)):
                vblk = v_sb[:, 128 * (2 * b + mc):128 * (2 * b + mc + 1)]
                mm(ps[0:64, :], vblk[:, 0:64], a[:, 0:512],
                   start=(mc == 0), stop=(mc == 1))
                mm(ps[64:128, :], vblk[:, 64:128], a[:, 512:1024],
                   start=(mc == 0), stop=(mc == 1))
            s = ppool.tile([128, 512], F16, tag="pairs")
            with tc.high_priority(offset=50):
                for half in (0, 256):
                    nc.vector.tensor_add(s[:, half:half + 256],
                                         ps[:, half:half + 256],
                                         sv_sb[:, 256 * b:256 * (b + 1)])
            return s

        def out_proj(it, b, pairs):
            # pairs[p//2] cols 256*(p%2)+: the (p, p+4) head pair block.
            for tcc in range(2):
                ps = psA.tile([128, 512], F32, tag="psA")
                for p in range(4):
                    pt = pairs[p // 2]
                    base = 256 * (p % 2) + 128 * tcc
                    mm(ps[:], pt[:, base:base + 128],
                       wo[p][:], start=(p == 0), stop=(p == 3))
                s = ypool.tile([128, 512], F16, tag="ys")
                nc.vector.tensor_copy(s[:], ps[:])
                nc.sync.dma_start(
                    y[2 * it + b, 128 * tcc:128 * (tcc + 1), :], s[:])

        # ---------- software pipeline ----------
        # q/k of iteration 0 up front (its v/sv fill iteration 0's own
        # tanh-latency gaps); per iteration the next iteration's
        # projections are interleaved into the attention's tanh gaps so
        # the PE never drains.
        qall0 = qpool.tile([128, 4, 512], F16, tag="qs")
        for j in range(4):
            proj_q(0, qall0, j)
        state[0] = (qall0, proj_k(0), None, None)
        for it in range(NPAIR):
            qall, ks, vs, svs = state[it]
            nxt = it + 1 if it + 1 < NPAIR else None
            if nxt is not None and nxt + 1 < NPAIR:
                fetch_x(nxt + 1)
            first = vs is None
            nqall = None
            if nxt is not None:
                nqall = qpool.tile([128, 4, 512], F16, tag="qs")

            # Each scores quad is immediately followed (in PE program order)
            # by an attn@v quad whose tanh inputs completed two quads ago:
            # pair-tiled matmul groups chain back-to-back, halving the
            # pair-group -> full-array transitions that pay an exposed
            # LDWEIGHTS (~100-250ns each).
            if first:
                aA = scores_quad(qall, ks, 0, 0)
                vs = proj_v(it)
                aC = scores_quad(qall, ks, 0, 2)
                svs = proj_sv(it, vs)
                if nxt is not None:
                    proj_q(nxt, nqall, 0)
                    proj_q(nxt, nqall, 1)
                p1 = attnv_quad(vs, svs, 0, *aA)
                p2 = attnv_quad(vs, svs, 0, *aC)
                out_proj(it, 0, [p1, p2])
                bA = scores_quad(qall, ks, 1, 0)
                proj_q(nxt, nqall, 2)
                bC = scores_quad(qall, ks, 1, 2)
                proj_q(nxt, nqall, 3)
                p1 = attnv_quad(vs, svs, 1, *bA)
                p2 = attnv_quad(vs, svs, 1, *bC)
                nks = proj_k(nxt)
                out_proj(it, 1, [p1, p2])
                nvs = proj_v(nxt)
                nsvs = proj_sv(nxt, nvs)
            elif nxt is not None:
                aA = scores_quad(qall, ks, 0, 0)
                proj_q(nxt, nqall, 0)
                proj_q(nxt, nqall, 1)
                aC = scores_quad(qall, ks, 0, 2)
                p1 = attnv_quad(vs, svs, 0, *aA)
                proj_q(nxt, nqall, 2)
                proj_q(nxt, nqall, 3)
                bA = scores_quad(qall, ks, 1, 0)
                p2 = attnv_quad(vs, svs, 0, *aC)
                nks = proj_k(nxt)
                out_proj(it, 0, [p1, p2])
                bC = scores_quad(qall, ks, 1, 2)
                p1 = attnv_quad(vs, svs, 1, *bA)
                nvs = proj_v(nxt)
                p2 = attnv_quad(vs, svs, 1, *bC)
                nsvs = proj_sv(nxt, nvs)
                out_proj(it, 1, [p1, p2])
            else:
                # last iteration: no next-iteration projections to fill the
                # tanh-latency gaps, so spread the output projections instead
                aA = scores_quad(qall, ks, 0, 0)
                aC = scores_quad(qall, ks, 0, 2)
                p1 = attnv_quad(vs, svs, 0, *aA)
                p2 = attnv_quad(vs, svs, 0, *aC)
                bA = scores_quad(qall, ks, 1, 0)
                out_proj(it, 0, [p1, p2])
                bC = scores_quad(qall, ks, 1, 2)
                p1 = attnv_quad(vs, svs, 1, *bA)
                p2 = attnv_quad(vs, svs, 1, *bC)
                out_proj(it, 1, [p1, p2])

            if nxt is not None:
                state[nxt] = (nqall, nks, nvs, nsvs)
            state[it] = None


def _get_runner():
    if "nc" not in _cached:
        _cached["nc"] = _build_nc()
    return _cached["nc"]


def _prep_inputs(x, sgr, Wq, Wk, Wv, Wo):
    f16 = np.float16
    x = np.asarray(x, dtype=np.float32)
    xb = x.reshape(BT, N, C)
    # head pair order [h0,h4 | h1,h5 | h2,h6 | h3,h7]
    perm = np.concatenate(
        [np.r_[64 * p:64 * (p + 1), 64 * (p + 4):64 * (p + 5)]
         for p in range(4)])
    Wqp = np.ascontiguousarray(np.asarray(Wq, dtype=np.float32)[:, perm]).astype(f16)
    Wop = np.ascontiguousarray(np.asarray(Wo, dtype=np.float32)[perm, :]).astype(f16)
    sgrT = np.ascontiguousarray(np.asarray(sgr, dtype=np.float32).T).astype(f16)
    Wk = np.ascontiguousarray(np.asarray(Wk, dtype=np.float32)).astype(f16)
    Wv = np.ascontiguousarray(np.asarray(Wv, dtype=np.float32)).astype(f16)

    in_maps = []
    for core in range(NCORES):
        xc = xb[PER_CORE * core: PER_CORE * (core + 1)]        # [16, 256, 512]
        xtc = xc.transpose(0, 2, 1)                            # [16, 512, 256]
        xarr = np.ascontiguousarray(
            xtc.reshape(NPAIR, 2, 4, 128, N)
               .transpose(0, 3, 2, 1, 4)
               .reshape(NPAIR, 128, 4, 512)).astype(f16)
        in_maps.append({
            "xT": xarr, "sgrT": sgrT, "Wqp": Wqp, "Wk": Wk,
            "Wv": Wv, "Wop": Wop,
        })
    return in_maps


def _run(x, sgr, Wq, Wk, Wv, Wo, trace=False, tmpdir=None):
    nc = _get_runner()
    in_maps = _prep_inputs(x, sgr, Wq, Wk, Wv, Wo)
    res = run_bass_kernel_spmd(nc, in_maps, list(range(NCORES)), trace=trace,
                               tmpdir=tmpdir)
    outs = [res.results[i]["y"] for i in range(NCORES)]
    full = np.concatenate(outs, axis=0).reshape(B, T, N, C).astype(np.float32)
    return full, res


def kernel(x, sgr, Wq, Wk, Wv, Wo):
    out, _ = _run(x, sgr, Wq, Wk, Wv, Wo, trace=False)
    return out


# revision 30
# speedup vs baseline: 1.0095x; 1.0095x over previous
"""GQA attention (tanh-score + static bias, no softmax) on 8 trn2 cores.

Reference shapes: x [4,32,256,512], H=8 heads, G=2 kv groups, D=64, N=256.
Strategy: data-parallel over the 128 (b,t) pairs -> 16 per core, zero
collectives.  All SBUF operands fp16 (1 cycle/row matmuls at any free size,
FWL weight loads, half the DMA of fp32), PSUM accumulation fp32.

Per (b,t):
  q^T = Wq^T x^T, k^T = Wk^T x^T   (feature-major, host-transposed x)
  v   = x @ Wv                      (token-major, lhsT = x^T chunks)
  sv^T = (sgr @ v)^T                (lhsT = v, rhs = sgr^T; both groups at once)
  scores^T[m,n] = k_g q_h^T         (K=64 contraction -> ROW-TILED: the two kv
                                     groups run concurrently in array halves;
                                     each mm streams TWO same-group heads in
                                     one N=512 rhs via a 2-block AP over the
                                     iteration's merged q tile)
  attn^T = tanh(scores^T * 0.125)   (ACT engine, scale fused, [128,1024]
                                     two-bank tiles to amortize overhead)
  out_h^T = v_g^T attn_h^T + sv^T   (COL-TILED: heads h and h+4 in col halves,
                                     zero wasted array; sv add fused into the
                                     PSUM->SBUF evacuation on DVE; quad outputs
                                     double-buffered through a PSUM pool shared
                                     with the v/sv banks)
  y = out @ Wo                      (token-major y, Wo host-permuted)

Host-side prep: x fp16 feature-major pre-tiled; Wq columns permuted to head
pairs (h, h+4) so each q tile holds one head per array half matching the k
group layout (no doubled Wk needed); Wo rows same permutation; sgr transposed.
"""

import os
import sys

import numpy as np

for _p in ("/opt/trn_rl_repo",):
    if _p not in sys.path and os.path.isdir(_p):
        sys.path.insert(0, _p)

import concourse.bass as bass
import concourse.tile as tile
from concourse import bacc, mybir
from concourse.bass_utils import run_bass_kernel_spmd

F32 = mybir.dt.float32
F16 = mybir.dt.float16

B, T, N, C = 4, 32, 256, 512
H, G, D = 8, 2, 64
NCORES = 8
BT = B * T                      # 128
PER_CORE = BT // NCORES         # 16
NPAIR = PER_CORE // 2           # 8 iterations of 2 (b,t) each
SCALE = D ** -0.5               # 0.125

_cached = {}


def _build_nc():
    """Build + lower the single-core SPMD program."""
    nc = bacc.Bacc("TRN2", target_bir_lowering=False, debug=False,
                   num_devices=NCORES)

    # DRAM I/O (per-core shard, host-side pre-arranged, fp16)
    # xT[i, p, c, 256*b + n] = x[bt=2i+b, tok=n, cin=128c+p]
    # weights declared chunk-major so each loads in ONE dma_start (each
    # dma_start costs ~600ns of HWDGE descriptor generation serialized on the
    # Sync engine; 4 separate chunk transfers per weight pushed the x[1]
    # prefetch completion out past 20us and re-throttled the PE clock)
    xT = nc.dram_tensor("xT", [NPAIR, 128, 4, 512], F16, kind="ExternalInput").ap()
    sgrT = nc.dram_tensor("sgrT", [2, 128, N], F16, kind="ExternalInput").ap()
    Wqp = nc.dram_tensor("Wqp", [4, 128, C], F16, kind="ExternalInput").ap()
    Wk = nc.dram_tensor("Wk", [4, 128, G * D], F16, kind="ExternalInput").ap()
    Wv = nc.dram_tensor("Wv", [4, 128, G * D], F16, kind="ExternalInput").ap()
    Wop = nc.dram_tensor("Wop", [4, 128, C], F16, kind="ExternalInput").ap()
    y = nc.dram_tensor("y", [PER_CORE, N, C], F16, kind="ExternalOutput").ap()

    with tile.TileContext(nc) as tc:
        _body(tc, xT, sgrT, Wqp, Wk, Wv, Wop, y)

    nc.compile()
    return nc


def _body(tc, xT, sgrT, Wqp, Wk, Wv, Wop, y):
    nc = tc.nc
    mm = nc.tensor.matmul
    import contextlib
    ctx = contextlib.ExitStack()
    with ctx:
        # SBUF state pools one deeper than strictly needed (bufs=3): the next
        # iteration's projection tiles become allocatable early, so the
        # scheduler always has ready PE work to fill attention-chain stalls.
        consts = ctx.enter_context(tc.tile_pool(name="consts", bufs=1))
        xpool = ctx.enter_context(tc.tile_pool(name="xt", bufs=4))
        qpool = ctx.enter_context(tc.tile_pool(name="qs", bufs=3))
        kpool = ctx.enter_context(tc.tile_pool(name="ks", bufs=3))
        vpool = ctx.enter_context(tc.tile_pool(name="vs", bufs=3))
        svpool = ctx.enter_context(tc.tile_pool(name="svs", bufs=3))
        apool = ctx.enter_context(tc.tile_pool(name="attn", bufs=8))
        ppool = ctx.enter_context(tc.tile_pool(name="pairs", bufs=6))
        ypool = ctx.enter_context(tc.tile_pool(name="ys", bufs=4))
        # PSUM: 8 banks of [128, 512] fp32.
        #   psA 2 banks (q / k / y cycling), psS 2x two-bank tiles (scores),
        #   psVP 2 banks shared by v, sv and the four attn@v quads so the
        #   quads double-buffer instead of serializing through one bank.
        psA = ctx.enter_context(
            tc.tile_pool(name="psA", bufs=2, space=bass.MemorySpace.PSUM))
        psS = ctx.enter_context(
            tc.tile_pool(name="psS", bufs=2, space=bass.MemorySpace.PSUM))
        psVP = ctx.enter_context(
            tc.tile_pool(name="psVP", bufs=2, space=bass.MemorySpace.PSUM))

        # ---- per-iteration x prefetch (issued ahead of the consts so the
        # first q matmuls have both operands as early as possible) ----
        xts = [None] * NPAIR

        def fetch_x(it, split=False):
            t = xpool.tile([128, 4, 512], F16, tag="xt")
            if split:
                # two transfers so the first q matmul (which only needs
                # chunk c=0) can start before the whole tile lands
                nc.sync.dma_start(t[:, 0:1, :], xT[it, :, 0:1, :])
                nc.sync.dma_start(t[:, 1:4, :], xT[it, :, 1:4, :])
            else:
                nc.sync.dma_start(t[:], xT[it])
            xts[it] = t

        fetch_x(0, split=True)

        # ---- resident constants, one dma_start each: the partition-major
        # src AP [(cols,128), (128*cols, nchunk), (1, cols)] lands chunk c at
        # tile free-slice [:, c, :].  Ordered by first use; x of iteration 1
        # is fetched before sgr/Wo since it is needed earlier. ----
        def fetch_const(dram, nchunk, cols):
            t = consts.tile([128, nchunk, cols], F16, tag=f"c{dram.tensor.name}")
            src = bass.AP(dram.tensor, 0,
                          [(cols, 128), (128 * cols, nchunk), (1, cols)])
            nc.sync.dma_start(t[:], src)
            return t

        wq = fetch_const(Wqp, 4, 512)
        wk = fetch_const(Wk, 4, 128)
        wv = fetch_const(Wv, 4, 128)
        fetch_x(1)
        sgt = fetch_const(sgrT, 2, 256)
        wo = fetch_const(Wop, 4, 512)

        # ---- warm-up while the first DMAs are in flight:
        #  * a tiny tanh so the ACT table set loads before the first real
        #    activation instead of serializing the first attention chain
        #  * dense dummy matmuls so the PE clock gate reaches 8/8 (needs
        #    ~3.4us of sustained busy) before real work starts.
        dummy = consts.tile([128, 512], F16, tag="dummy")
        nc.gpsimd.memset(dummy[:], 0.0)
        dtanh = consts.tile([128, 16], F16, tag="dtanh")
        nc.scalar.activation(dtanh[:], dummy[:, 0:16],
                             mybir.ActivationFunctionType.Tanh, scale=SCALE)
        wps = psVP.tile([128, 512], F32, tag="psVP")
        for _ in range(10):
            mm(wps[:], dummy[:, 0:128], dummy[:], start=True, stop=True)

        # per-iteration state handed between pipeline stages
        state = [None] * NPAIR

        # ---------- stage A pieces (projections for iteration it) ----------
        def proj_q(it, qall, j):
            # writes the j-th 512-col chunk of the iteration's merged q tile;
            # consecutive psA groups alternate ACT/DVE evacuation so both
            # banks drain concurrently (halves the bank-recycle stall)
            xt = xts[it]
            ps = psA.tile([128, 512], F32, tag="psA")
            for c in range(4):
                mm(ps[:], wq[:, c, 128 * j:128 * (j + 1)], xt[:, c, :],
                   start=(c == 0), stop=(c == 3))
            nc.vector.tensor_copy(qall[:, j, :], ps[:])

        def proj_k(it):
            xt = xts[it]
            ps = psA.tile([128, 512], F32, tag="psA")
            for c in range(4):
                mm(ps[:], wk[:, c, :], xt[:, c, :],
                   start=(c == 0), stop=(c == 3))
            s = kpool.tile([128, 512], F16, tag="ks")
            nc.vector.tensor_copy(s[:], ps[:])
            return s

        def proj_v(it):
            # token-major v: block 2b+mc at cols 128*(2b+mc) holds
            # v[tok chunk mc of bt b, (g0 d | g1 d)]
            xt = xts[it]
            ps = psVP.tile([128, 512], F32, tag="psVP")
            for blk in range(4):
                b, mcc = blk // 2, blk % 2
                off = 256 * b + 128 * mcc
                for c in range(4):
                    mm(ps[:, 128 * blk:128 * (blk + 1)],
                       xt[:, c, off:off + 128], wv[:, c, :],
                       start=(c == 0), stop=(c == 3))
            s = vpool.tile([128, 512], F16, tag="vs")
            nc.vector.tensor_copy(s[:], ps[:])
            return s

        def proj_sv(it, v_sb):
            # sv^T[dpair, n] for both bt: (sgr@v_g)^T rows 64g:64g+64
            ps = psVP.tile([128, 512], F32, tag="psVP")
            for b in range(2):
                for mc in range(2):
                    mm(ps[:, 256 * b:256 * (b + 1)],
                       v_sb[:, 128 * (2 * b + mc):128 * (2 * b + mc + 1)],
                       sgt[:, mc, :], start=(mc == 0), stop=(mc == 1))
            s = svpool.tile([128, 512], F32, tag="svs")
            nc.vector.tensor_copy(s[:], ps[:])
            return s

        # ---------- stage B pieces (attention for iteration it, bt b) ------
        def scores_quad(qall, ks, b, j0):
            # one two-bank psum tile per m-chunk holding all four heads of
            # the quad: [h_j0 | h_j0+1 | h_j0+4 | h_j0+5] (one 256-col block
            # each).  Each mm streams both same-group heads in one N=512 rhs
            # (2-block AP over the merged q tile); the two group mms
            # alternate array row halves 0:64 / 64:128 and dual-issue.
            outs = []
            for mc in range(2):
                off = 256 * b + 128 * mc
                ps = psS.tile([128, 1024], F32, tag="psS")
                # the scores->tanh loop through the two psS tiles is the
                # serial heart of the kernel: prioritize it so a freed psS
                # tile is refilled ahead of any waiting projection matmuls
                with tc.high_priority(offset=40):
                    mm(ps[:, 0:512],
                       ks[0:64, off:off + 128],
                       qall[0:64, j0:j0 + 2, 256 * b:256 * (b + 1)],
                       start=True, stop=True)
                    mm(ps[:, 512:1024],
                       ks[64:128, off:off + 128],
                       qall[64:128, j0:j0 + 2, 256 * b:256 * (b + 1)],
                       start=True, stop=True)
                    a = apool.tile([128, 1024], F16, tag="attn")
                    nc.scalar.activation(a[:], ps[:],
                                         mybir.ActivationFunctionType.Tanh,
                                         scale=SCALE)
                outs.append(a)
            return outs

        def attnv_quad(v_sb, sv_sb, b, a0, a1):
            # col-tiled fat matmuls: rows 0:64 = group-0 head pair, rows
            # 64:128 = group-1 head pair, each a single 512-free matmul per
            # m-chunk covering both heads (adjacent in the a tile).  Output
            # quadrants: (0:64, 0:256)=h_even^T g0, (0:64, 256:512)=h_odd^T,
            # (64:128, *) same for group 1 -> col 256-blocks are the
            # (h, h+4) pairs that Wop expects.
            ps = psVP.tile([128, 512], F32, tag="psVP")
            for mc, a in ((0, a0), (1, a1)):
                vblk = v_sb[:, 128 * (2 * b + mc):128 * (2 * b + mc + 1)]
                mm(ps[0:64, :], vblk[:, 0:64], a[:, 0:512],
                   start=(mc == 0), stop=(mc == 1))
                mm(ps[64:128, :], vblk[:, 64:128], a[:, 512:1024],
                   start=(mc == 0), stop=(mc == 1))
            s = ppool.tile([128, 512], F16, tag="pairs")
            with tc.high_priority(offset=50):
                for half in (0, 256):
                    nc.vector.tensor_add(s[:, half:half + 256],
                                         ps[:, half:half + 256],
                                         sv_sb[:, 256 * b:256 * (b + 1)])
            return s

        def out_proj(it, b, pairs):
            # pairs[p//2] cols 256*(p%2)+: the (p, p+4) head pair block.
            for tcc in range(2):
                ps = psA.tile([128, 512], F32, tag="psA")
                for p in range(4):
                    pt = pairs[p // 2]
                    base = 256 * (p % 2) + 128 * tcc
                    mm(ps[:], pt[:, base:base + 128],
                       wo[:, p, :], start=(p == 0), stop=(p == 3))
                s = ypool.tile([128, 512], F16, tag="ys")
                nc.vector.tensor_copy(s[:], ps[:])
                nc.sync.dma_start(
                    y[2 * it + b, 128 * tcc:128 * (tcc + 1), :], s[:])

        # ---------- software pipeline ----------
        # q/k of iteration 0 up front (its v/sv fill iteration 0's own
        # tanh-latency gaps); per iteration the next iteration's
        # projections are interleaved into the attention's tanh gaps so
        # the PE never drains.
        qall0 = qpool.tile([128, 4, 512], F16, tag="qs")
        for j in range(4):
            proj_q(0, qall0, j)
        state[0] = (qall0, proj_k(0), None, None)
        for it in range(NPAIR):
            qall, ks, vs, svs = state[it]
            nxt = it + 1 if it + 1 < NPAIR else None
            if nxt is not None and nxt + 1 < NPAIR:
                fetch_x(nxt + 1)
            first = vs is None
            nqall = None
            if nxt is not None:
                nqall = qpool.tile([128, 4, 512], F16, tag="qs")

            # Each scores quad is immediately followed (in PE program order)
            # by an attn@v quad whose tanh inputs completed two quads ago:
            # pair-tiled matmul groups chain back-to-back, halving the
            # pair-group -> full-array transitions that pay an exposed
            # LDWEIGHTS (~100-250ns each).
            if first:
                aA = scores_quad(qall, ks, 0, 0)
                vs = proj_v(it)
                aC = scores_quad(qall, ks, 0, 2)
                svs = proj_sv(it, vs)
                if nxt is not None:
                    proj_q(nxt, nqall, 0)
                    proj_q(nxt, nqall, 1)
                p1 = attnv_quad(vs, svs, 0, *aA)
                p2 = attnv_quad(vs, svs, 0, *aC)
                out_proj(it, 0, [p1, p2])
                bA = scores_quad(qall, ks, 1, 0)
                proj_q(nxt, nqall, 2)
                bC = scores_quad(qall, ks, 1, 2)
                proj_q(nxt, nqall, 3)
                p1 = attnv_quad(vs, svs, 1, *bA)
                p2 = attnv_quad(vs, svs, 1, *bC)
                nks = proj_k(nxt)
                out_proj(it, 1, [p1, p2])
                nvs = proj_v(nxt)
                nsvs = proj_sv(nxt, nvs)
            elif nxt is not None:
                aA = scores_quad(qall, ks, 0, 0)
                proj_q(nxt, nqall, 0)
                proj_q(nxt, nqall, 1)
                aC = scores_quad(qall, ks, 0, 2)
                p1 = attnv_quad(vs, svs, 0, *aA)
                proj_q(nxt, nqall, 2)
                proj_q(nxt, nqall, 3)
                bA = scores_quad(qall, ks, 1, 0)
                p2 = attnv_quad(vs, svs, 0, *aC)
                nks = proj_k(nxt)
                out_proj(it, 0, [p1, p2])
                bC = scores_quad(qall, ks, 1, 2)
                p1 = attnv_quad(vs, svs, 1, *bA)
                nvs = proj_v(nxt)
                p2 = attnv_quad(vs, svs, 1, *bC)
                nsvs = proj_sv(nxt, nvs)
                out_proj(it, 1, [p1, p2])
            else:
                # last iteration: no next-iteration projections to fill the
                # tanh-latency gaps, so spread the output projections instead
                aA = scores_quad(qall, ks, 0, 0)
                aC = scores_quad(qall, ks, 0, 2)
                p1 = attnv_quad(vs, svs, 0, *aA)
                p2 = attnv_quad(vs, svs, 0, *aC)
                bA = scores_quad(qall, ks, 1, 0)
                out_proj(it, 0, [p1, p2])
                bC = scores_quad(qall, ks, 1, 2)
                p1 = attnv_quad(vs, svs, 1, *bA)
                p2 = attnv_quad(vs, svs, 1, *bC)
                out_proj(it, 1, [p1, p2])

            if nxt is not None:
                state[nxt] = (nqall, nks, nvs, nsvs)
            state[it] = None


def _get_runner():
    if "nc" not in _cached:
        _cached["nc"] = _build_nc()
    return _cached["nc"]


def _prep_inputs(x, sgr, Wq, Wk, Wv, Wo):
    f16 = np.float16
    x = np.asarray(x, dtype=np.float32)
    xb = x.reshape(BT, N, C)
    # head pair order [h0,h4 | h1,h5 | h2,h6 | h3,h7]
    perm = np.concatenate(
        [np.r_[64 * p:64 * (p + 1), 64 * (p + 4):64 * (p + 5)]
         for p in range(4)])
    Wqp = np.ascontiguousarray(
        np.asarray(Wq, dtype=np.float32)[:, perm]).astype(f16).reshape(4, 128, 512)
    Wop = np.ascontiguousarray(
        np.asarray(Wo, dtype=np.float32)[perm, :]).astype(f16).reshape(4, 128, 512)
    sgrT = np.ascontiguousarray(
        np.asarray(sgr, dtype=np.float32).T).astype(f16).reshape(2, 128, 256)
    Wk = np.ascontiguousarray(
        np.asarray(Wk, dtype=np.float32)).astype(f16).reshape(4, 128, 128)
    Wv = np.ascontiguousarray(
        np.asarray(Wv, dtype=np.float32)).astype(f16).reshape(4, 128, 128)

    in_maps = []
    for core in range(NCORES):
        xc = xb[PER_CORE * core: PER_CORE * (core + 1)]        # [16, 256, 512]
        xtc = xc.transpose(0, 2, 1)                            # [16, 512, 256]
        xarr = np.ascontiguousarray(
            xtc.reshape(NPAIR, 2, 4, 128, N)
               .transpose(0, 3, 2, 1, 4)
               .reshape(NPAIR, 128, 4, 512)).astype(f16)
        in_maps.append({
            "xT": xarr, "sgrT": sgrT, "Wqp": Wqp, "Wk": Wk,
            "Wv": Wv, "Wop": Wop,
        })
    return in_maps


def _run(x, sgr, Wq, Wk, Wv, Wo, trace=False, tmpdir=None):
    nc = _get_runner()
    in_maps = _prep_inputs(x, sgr, Wq, Wk, Wv, Wo)
    res = run_bass_kernel_spmd(nc, in_maps, list(range(NCORES)), trace=trace,
                               tmpdir=tmpdir)
    outs = [res.results[i]["y"] for i in range(NCORES)]
    full = np.concatenate(outs, axis=0).reshape(B, T, N, C).astype(np.float32)
    return full, res


def kernel(x, sgr, Wq, Wk, Wv, Wo):
    out, _ = _run(x, sgr, Wq, Wk, Wv, Wo, trace=False)
    return out
